# revision 1
# baseline (speedup 1.0000x reference)
"""Trainium2 Bass kernel for nn_DLGeneEmbeddings.

Math (separable linear):
    y[b, j] = w_x * x[b, j] + (nongene[b] . W_ng + bias) + (emb[j] . W_e)
with
    nongene = x[:, G:G+64], W = [W_ng(64) | w_x(1) | W_e(32)].

Sharding: gene-parallel across 8 cores; each core handles a 2500-gene
column slice for the full 1024-row batch. The embedding table shards
naturally with the genes; the tiny fc weights are replicated (the host
pre-broadcasts/packs them -- pure layout, no math).

The tolerance budget (rel err < 2e-2) is spent on HBM traffic:
  x gene columns are fed as fp8 e3m4 (4 mantissa bits, |x| <= 5.5 fits
  the +-15.5 range; measured end-to-end rel err ~6e-3), the nongene
  columns and embedding slice as bf16, and y is stored as bf16 and
  upcast on the host. Per-core traffic drops 23.1 MB -> ~8.1 MB, which
  at the ~360 GB/s per-core HBM limit is ~22.5 us of unavoidable DMA.

Per-core device kernel, engineered so the serialized DMA stream is the
only critical resource:
  PE:     everything reduction-shaped, via the broadcast-row trick:
          - gene term: lhsT = [W_e | b]-broadcast [33, 128], rhs =
            [embT ; ones] [33, 2500] => PSUM[m, j] = gene[j] + b on
            every partition m (one matmul + bf16 copy per PSUM bank)
          - ng term: lhsT = xngT block [64, 128], rhs = W_ng column
            [64, 1] => PSUM[p, a] = nongene[a*128+p] . W_ng
  DVE:    tiny PSUM->SBUF copies (+ w_x bf16->f32); main y += grow
          adds (bf16 2x mode)
  ACT:    y[:, :SPLIT] = Identity(x * w_x + ngb[a]), fp8 -> bf16; a
          t~0 dummy op hoists the activation-table load
  Pool:   y[:, SPLIT:] = x * w_x + ngb[a] via tensor_scalar (the
          scale-add splits across ACT and Pool so neither gates)
  DMA:    three loads (xngT+W_ng+w_x packed; embT+W_e+b packed; 8 x
          row blocks) up front on the SP HWDGE ring -- x blocks all
          get their own buffer, so no cross-engine waits ahead of any
          load -- then the 16 y half-block stores on the same ring in
          dependency order. The DMA engines run dense start to end.

(NB: tensor_tensor_reduce crashes the HW exec unit -- avoid it.)
"""

import numpy as np
import ml_dtypes
from contextlib import ExitStack

import concourse.bass as bass
import concourse.bacc as bacc
import concourse.tile as tile
from concourse import mybir
from concourse.bass_utils import run_bass_kernel_spmd

F32 = mybir.dt.float32
BF16 = mybir.dt.bfloat16
FP8 = mybir.dt.float8e3

NP_BF16 = ml_dtypes.bfloat16
NP_FP8 = ml_dtypes.float8_e3m4

B = 1024
G = 20000
DNG = 64
E = 32
FC_IN = DNG + 1 + E       # 97
NCORES = 8
GC = G // NCORES          # 2500 gene columns per core
PB = 128                  # SBUF partitions
RB = B // PB              # 8 row blocks per core
SPLIT = 1536              # ACT handles [0:SPLIT), Pool [SPLIT:GC) -- the
                          # Pool tensor op has ~790 ns more fixed cost

BANK = 512                # f32 columns per PSUM bank
NBANK = (GC + BANK - 1) // BANK


def build_kernel(nc: bass.Bass, repeat: int = 1):
    xgd = nc.dram_tensor("xg", [B, GC], FP8, kind="ExternalInput").ap()
    xngTd = nc.dram_tensor("xngT", [DNG, B + 3], BF16, kind="ExternalInput").ap()
    embTd = nc.dram_tensor("embT", [E + 1, GC + PB], BF16, kind="ExternalInput").ap()
    ysd = nc.dram_tensor("ys", [B, GC], BF16, kind="ExternalOutput").ap()

    with tile.TileContext(nc) as tc, ExitStack() as ctx:
        const = ctx.enter_context(tc.tile_pool(name="const", bufs=1))
        psum = ctx.enter_context(tc.tile_pool(name="psum", bufs=1, space="PSUM"))
        xpool = ctx.enter_context(tc.tile_pool(name="xpool", bufs=RB))
        ypool = ctx.enter_context(tc.tile_pool(name="ypool", bufs=RB))

        # ---- dummy activation: hoists LoadActFuncSet to t~0 ----
        zin = const.tile([1, 2], F32)
        nc.gpsimd.memset(zin, 0.0)
        zout = const.tile([1, 2], F32)
        nc.scalar.activation(
            out=zout, in_=zin, func=mybir.ActivationFunctionType.Identity
        )

        # ---- loads, all on the SP HWDGE ring, nothing blocking ----
        # xngT packs W_ng (col B) and w_x (col B+1); w_x is broadcast
        # to all 128 partitions below via a 1x1 PE matmul against the
        # embTa ones-row.
        xnga = const.tile([DNG, B + 3], BF16)
        nc.sync.dma_start(out=xnga, in_=xngTd)
        xngT = xnga[:, 0:B + 1]
        wngcol = xnga[:, B:B + 1]           # [64, 1] = W_ng

        embTa = const.tile([E + 1, GC + PB], BF16)
        nc.sync.dma_start(out=embTa, in_=embTd)
        embT = embTa[:, 0:GC]               # [33, 2500] = [embT ; ones]
        web = embTa[:, GC:GC + PB]          # [33, 128]  = [W_e | b] bcast

        x_ts = []
        for a in range(RB):
            x_t = xpool.tile([PB, GC], FP8, tag="x")
            x_ts.append(x_t)
        for a in range(RB):
            nc.sync.dma_start(out=x_ts[a], in_=xgd[a * PB:(a + 1) * PB, :])

        # w_x broadcast across partitions: ones[1,128]^T @ wx[1,1]
        # (the ones cell lives in xnga col B+2, broadcast along free)
        wxp = psum.tile([PB, 1], F32, tag="wx")
        nc.tensor.matmul(
            wxp,
            xnga[0:1, B + 2:B + 3].to_broadcast([1, PB]),
            xnga[0:1, B + 1:B + 2],
            start=True,
            stop=True,
        )
        wxc = const.tile([PB, 1], F32)
        nc.vector.tensor_copy(wxc, wxp)



        # ---- ng term on PE: ngp[p, a] = nongene[a*128+p] . W_ng ----
        ngp = psum.tile([PB, RB], F32, tag="ng")
        for a in range(RB):
            nc.tensor.matmul(
                ngp[:, a:a + 1],
                xngT[:, a * PB:(a + 1) * PB],
                wngcol,
                start=True,
                stop=True,
            )
        ngb = const.tile([PB, RB], F32)
        nc.vector.tensor_copy(ngb, ngp)

        # ---- gene term (+ fc bias): matmul + bf16 copy per bank ----
        # One PSUM tile per bank: a shared tile would serialize matmul q
        # against the copy of bank q-1 through a false WAR dependency.
        grow = const.tile([PB, GC], BF16)
        for q in range(NBANK):
            c0 = q * BANK
            cw = min(BANK, GC - c0)
            gps = psum.tile([PB, BANK], F32, tag=f"g{q}")
            nc.tensor.matmul(
                gps[:, 0:cw],
                web,
                embT[:, c0:c0 + cw],
                start=True,
                stop=True,
            )
            nc.vector.tensor_copy(grow[:, c0:c0 + cw], gps[:, 0:cw])

        # ---- main stream over 8 row blocks, two-phase issue ----
        # Phase 1 issues every activation / pool scale-add; phase 2
        # issues adds + stores. Execution is still dataflow-ordered by
        # semaphores, but no store ever sits AHEAD of a compute op in
        # an engine's in-order queue.
        lo = slice(0, SPLIT)
        hi = slice(SPLIT, GC)
        for r in range(repeat):
            y_ts = []
            for a in range(RB):
                r0 = a * PB
                if repeat > 1 and r > 0:
                    x_t = xpool.tile([PB, GC], FP8, tag="x")
                    nc.sync.dma_start(out=x_t, in_=xgd[r0:r0 + PB, :])
                else:
                    x_t = x_ts[a]
                y_t = ypool.tile([PB, GC], BF16, tag="y")
                y_ts.append(y_t)
                nc.scalar.activation(
                    out=y_t[:, lo],
                    in_=x_t[:, lo],
                    func=mybir.ActivationFunctionType.Identity,
                    bias=ngb[:, a:a + 1],
                    scale=wxc,
                )
                nc.gpsimd.tensor_scalar(
                    out=y_t[:, hi],
                    in0=x_t[:, hi],
                    scalar1=wxc,
                    scalar2=ngb[:, a:a + 1],
                    op0=mybir.AluOpType.mult,
                    op1=mybir.AluOpType.add,
                )
            for a in range(RB):
                r0 = a * PB
                y_t = y_ts[a]
                for h, sl in enumerate((lo, hi)):
                    nc.vector.tensor_add(y_t[:, sl], y_t[:, sl], grow[:, sl])
                    if a < 2:
                        nc.sync.dma_start(out=ysd[r0:r0 + PB, sl], in_=y_t[:, sl])
                if a >= 2:
                    nc.sync.dma_start(out=ysd[r0:r0 + PB, :], in_=y_t)


def make_nc(repeat: int = 1) -> bacc.Bacc:
    nc = bacc.Bacc("TRN2", debug=False, num_devices=NCORES)
    build_kernel(nc, repeat=repeat)
    nc.compile()  # legalizes sync waits (<=1 per instruction on TRN2)
    return nc


def prep_inputs(inputs) -> list:
    """Shard + downcast the full inputs into per-core in_maps."""
    x = np.asarray(inputs["x"], dtype=np.float32)
    emb = np.asarray(inputs["emb"], dtype=np.float32)
    W = np.asarray(inputs["W"], dtype=np.float32).reshape(FC_IN)
    b = float(np.asarray(inputs["b"], dtype=np.float32).reshape(()))

    # xngT[k, r] = x[r, G+k]; col B = W_ng; col B+1 = w_x; col B+2 = 1
    xngT = np.empty((DNG, B + 3), dtype=np.float32)
    xngT[:, 0:B] = x[:, G:].T
    xngT[:, B] = W[0:DNG]
    xngT[:, B + 1] = W[DNG]
    xngT[:, B + 2] = 1.0
    xngT = xngT.astype(NP_BF16)

    # aux block shared by all cores: [W_e | b] broadcast to 128 cols,
    # with the ones row that turns the bias into part of the gene matmul
    aux = np.empty((E + 1, PB), dtype=np.float32)
    aux[0:E, :] = W[DNG + 1:FC_IN, None]
    aux[E, :] = b

    in_maps = []
    for c in range(NCORES):
        sl = slice(c * GC, (c + 1) * GC)
        embTa = np.empty((E + 1, GC + PB), dtype=np.float32)
        embTa[0:E, 0:GC] = emb[sl].T
        embTa[E, 0:GC] = 1.0
        embTa[:, GC:] = aux
        in_maps.append({
            "xg": np.ascontiguousarray(x[:, sl]).astype(NP_FP8),
            "xngT": xngT,
            "embT": embTa.astype(NP_BF16),
        })
    return in_maps


def kernel(**inputs) -> np.ndarray:
    nc = make_nc()
    in_maps = prep_inputs(inputs)
    res = run_bass_kernel_spmd(nc, in_maps, core_ids=list(range(NCORES)))
    return np.concatenate(
        [np.asarray(r["ys"]).astype(np.float32) for r in res.results], axis=1
    )



# revision 2
# speedup vs baseline: 1.1476x; 1.1476x over previous
"""Trainium2 Bass kernel for nn_DLGeneEmbeddings — v2 (DMA restructure).

Math (separable linear):
    y[b, j] = w_x * x[b, j] + (nongene[b] . W_ng + bias) + (emb[j] . W_e)

Sharding: gene-parallel across 8 cores; each core handles a 2500-gene
column slice for the full 1024-row batch.

v2 vs baseline: the one-shot latency is dominated by the serialized DMA
stream (27 small DMAs on one HWDGE ring).  v2 cuts this to 9 DMAs spread
over all three DMA queues (qSPDynamicHW / qActDynamicHW / SWDGE):
  - 1 packed const load on the gpsimd (SWDGE) queue,
  - 4 x quarter-loads [128, 5000] fp8, alternating sync/act queues,
  - 4 y pair-stores [128, 5000] bf16 split sync/act/gpsimd queues,
    issued only at queue positions that never stall a compute engine.
x and y use a col-block = row-block packed DRAM layout ([128, 8*2500])
so row blocks are column ranges of one 128-partition tensor (pure host
layout transform).
"""

import numpy as np
import ml_dtypes
from contextlib import ExitStack

import concourse.bass as bass
import concourse.bacc as bacc
import concourse.tile as tile
from concourse import mybir
from concourse.bass_utils import run_bass_kernel_spmd

F32 = mybir.dt.float32
BF16 = mybir.dt.bfloat16
FP8 = mybir.dt.float8e3

NP_BF16 = ml_dtypes.bfloat16
NP_FP8 = ml_dtypes.float8_e3m4

B = 1024
G = 20000
DNG = 64
E = 32
FC_IN = DNG + 1 + E       # 97
NCORES = 8
GC = G // NCORES          # 2500 gene columns per core
PB = 128                  # SBUF partitions
RB = B // PB              # 8 row blocks per core
NQ = 4                    # x load quarters / y store pairs
QW = RB // NQ * GC        # 5000 cols per quarter/pair
SPLIT = 1536              # ACT handles [0:SPLIT), Pool [SPLIT:GC) per block

BANK = 512                # f32 columns per PSUM bank
NBANK = (GC + BANK - 1) // BANK

# const pack layout: [64, CW] bf16
#   rows 0:33, cols 0:GC        = [embT ; ones]
#   rows 0:33, cols GC:GC+PB    = [W_e | b] broadcast
#   rows 0:64, cols CX:CX+B     = xng.T
#   rows 0:64, col  CX+B        = W_ng
#   rows 0:64, col  CX+B+1      = w_x
#   rows 0:64, col  CX+B+2      = 1.0
CX = GC + PB              # 2628
CW = CX + B + 3           # 3655


def build_kernel(nc: bass.Bass, repeat: int = 1, serial: bool = False):
    xgd = nc.dram_tensor("xg", [PB, RB * GC], FP8, kind="ExternalInput").ap()
    cpkd = nc.dram_tensor("cpk", [DNG, CW], BF16, kind="ExternalInput").ap()
    ysd = nc.dram_tensor("ys", [PB, RB * GC], BF16, kind="ExternalOutput").ap()

    with tile.TileContext(nc) as tc, ExitStack() as ctx:
        const = ctx.enter_context(tc.tile_pool(name="const", bufs=1))
        psum = ctx.enter_context(tc.tile_pool(name="psum", bufs=1, space="PSUM"))
        xpool = ctx.enter_context(tc.tile_pool(name="xpool", bufs=NQ))
        ypool = ctx.enter_context(tc.tile_pool(name="ypool", bufs=NQ))

        # ---- dummy activation: hoists LoadActFuncSet to t~0 ----
        zin = const.tile([1, 2], F32)
        nc.gpsimd.memset(zin, 0.0)
        zout = const.tile([1, 2], F32)
        nc.scalar.activation(
            out=zout, in_=zin, func=mybir.ActivationFunctionType.Identity
        )

        # ---- const load on the SWDGE queue; x quarters on the two ----
        # ---- HWDGE queues, two each, so all three queues pull at t=0 ----
        cpk = const.tile([DNG, CW], BF16)
        nc.gpsimd.dma_start(out=cpk, in_=cpkd)
        embT = cpk[0:E + 1, 0:GC]               # [33, 2500] = [embT ; ones]
        web = cpk[0:E + 1, GC:GC + PB]          # [33, 128]  = [W_e | b] bcast
        xngT = cpk[:, CX:CX + B]                # [64, 1024] = xng.T
        wngcol = cpk[:, CX + B:CX + B + 1]      # [64, 1]    = W_ng

        gate = const.tile([1, NQ], BF16)        # serial-mode barrier tile

        def issue_x_loads(first: bool):
            x_ts = []
            for q in range(NQ):
                x_t = xpool.tile([PB, QW], FP8, tag="x")
                x_ts.append(x_t)
            if serial and not first:
                # gate: strided read touching every stored region, then a
                # tiny DVE splash into each x tile corner.  Forces repeat
                # r's loads to wait for r-1's stores (one-shot latency
                # approximation for the repeat-slope measurement).
                nc.sync.dma_start(
                    out=gate, in_=ysd[0:1, QW - 2:RB * GC:QW]
                )
                for q in range(NQ):
                    nc.vector.tensor_copy(x_ts[q][0:1, 0:NQ], gate)
            for q in range(NQ):
                eng = nc.sync if q % 2 == 0 else nc.scalar
                eng.dma_start(
                    out=x_ts[q], in_=xgd[:, q * QW:(q + 1) * QW]
                )
            return x_ts

        x_ts = issue_x_loads(first=True)

        # w_x broadcast across partitions: ones[1,128]^T @ wx[1,1]
        wxp = psum.tile([PB, 1], F32, tag="wx")
        nc.tensor.matmul(
            wxp,
            cpk[0:1, CX + B + 2:CX + B + 3].to_broadcast([1, PB]),
            cpk[0:1, CX + B + 1:CX + B + 2],
            start=True,
            stop=True,
        )
        wxc = const.tile([PB, 1], F32)
        nc.vector.tensor_copy(wxc, wxp)

        # ---- ng term on PE: ngp[p, a] = nongene[a*128+p] . W_ng ----
        ngp = psum.tile([PB, RB], F32, tag="ng")
        for a in range(RB):
            nc.tensor.matmul(
                ngp[:, a:a + 1],
                xngT[:, a * PB:(a + 1) * PB],
                wngcol,
                start=True,
                stop=True,
            )
        ngb = const.tile([PB, RB], F32)
        nc.vector.tensor_copy(ngb, ngp)

        # ---- gene term (+ fc bias): matmul + bf16 copy per bank ----
        grow = const.tile([PB, GC], BF16)
        for q in range(NBANK):
            c0 = q * BANK
            cw = min(BANK, GC - c0)
            gps = psum.tile([PB, BANK], F32, tag=f"g{q}")
            nc.tensor.matmul(
                gps[:, 0:cw],
                web,
                embT[:, c0:c0 + cw],
                start=True,
                stop=True,
            )
            nc.vector.tensor_copy(grow[:, c0:c0 + cw], gps[:, 0:cw])

        # ---- main stream: 8 row blocks in 4 quarter tiles ----
        # Block a lives in x_ts[a//2][:, (a%2)*GC :], y pair tiles hold
        # blocks (2k, 2k+1).  ACT does cols [0:SPLIT), Pool the rest,
        # DVE adds grow.  Stores: pair 0,2 -> sync queue (idle engine),
        # pair 1 -> act queue after its compute, pair 3 -> gpsimd queue
        # after its compute.
        lo = slice(0, SPLIT)
        hi = slice(SPLIT, GC)
        for r in range(repeat):
            if r > 0:
                x_ts = issue_x_loads(first=False)
            y_ts = []
            for k in range(NQ):
                y_t = ypool.tile([PB, QW], BF16, tag="y")
                y_ts.append(y_t)
            # phase 1: per-block scale-add on ACT (lo) and Pool (hi)
            for a in range(RB):
                x_blk = x_ts[a // 2][:, (a % 2) * GC:(a % 2 + 1) * GC]
                y_blk = y_ts[a // 2][:, (a % 2) * GC:(a % 2 + 1) * GC]
                nc.scalar.activation(
                    out=y_blk[:, lo],
                    in_=x_blk[:, lo],
                    func=mybir.ActivationFunctionType.Identity,
                    bias=ngb[:, a:a + 1],
                    scale=wxc,
                )
                nc.gpsimd.tensor_scalar(
                    out=y_blk[:, hi],
                    in0=x_blk[:, hi],
                    scalar1=wxc,
                    scalar2=ngb[:, a:a + 1],
                    op0=mybir.AluOpType.mult,
                    op1=mybir.AluOpType.add,
                )
            # phase 2: grow adds on DVE + stores
            for k in range(NQ):
                y_t = y_ts[k]
                for e in range(2):
                    for sl in (lo, hi):
                        dst = slice(e * GC + sl.start, e * GC + sl.stop)
                        nc.vector.tensor_add(y_t[:, dst], y_t[:, dst], grow[:, sl])
                eng = nc.sync if k % 2 == 0 else (nc.scalar if k == 1 else nc.gpsimd)
                eng.dma_start(out=ysd[:, k * QW:(k + 1) * QW], in_=y_t)


def make_nc(repeat: int = 1, serial: bool = False) -> bacc.Bacc:
    nc = bacc.Bacc("TRN2", debug=False, num_devices=NCORES)
    build_kernel(nc, repeat=repeat, serial=serial)
    nc.compile()
    return nc


def prep_inputs(inputs) -> list:
    """Shard + downcast + repack the full inputs into per-core in_maps."""
    x = np.asarray(inputs["x"], dtype=np.float32)
    emb = np.asarray(inputs["emb"], dtype=np.float32)
    W = np.asarray(inputs["W"], dtype=np.float32).reshape(FC_IN)
    b = float(np.asarray(inputs["b"], dtype=np.float32).reshape(()))

    base = np.zeros((DNG, CW), dtype=np.float32)
    base[:, CX:CX + B] = x[:, G:].T
    base[:, CX + B] = W[0:DNG]
    base[:, CX + B + 1] = W[DNG]
    base[:, CX + B + 2] = 1.0
    base[0:E, GC:GC + PB] = W[DNG + 1:FC_IN, None]
    base[E, GC:GC + PB] = b

    in_maps = []
    for c in range(NCORES):
        sl = slice(c * GC, (c + 1) * GC)
        cpk = base.copy()
        cpk[0:E, 0:GC] = emb[sl].T
        cpk[E, 0:GC] = 1.0
        xg = (
            np.ascontiguousarray(x[:, sl])
            .reshape(RB, PB, GC)
            .transpose(1, 0, 2)
            .reshape(PB, RB * GC)
        )
        in_maps.append({
            "xg": np.ascontiguousarray(xg).astype(NP_FP8),
            "cpk": cpk.astype(NP_BF16),
        })
    return in_maps


def unshard(res_core: np.ndarray) -> np.ndarray:
    """[128, 8*2500] packed -> [1024, 2500] row-major (pure layout)."""
    return (
        np.asarray(res_core)
        .reshape(PB, RB, GC)
        .transpose(1, 0, 2)
        .reshape(B, GC)
        .astype(np.float32)
    )


def kernel(**inputs) -> np.ndarray:
    nc = make_nc()
    in_maps = prep_inputs(inputs)
    res = run_bass_kernel_spmd(nc, in_maps, core_ids=list(range(NCORES)))
    return np.concatenate(
        [unshard(r["ys"]) for r in res.results], axis=1
    )


# revision 3
# speedup vs baseline: 1.2232x; 1.0659x over previous
"""Trainium2 Bass kernel for nn_DLGeneEmbeddings.

Math (separable linear):
    y[b, j] = w_x * x[b, j] + (nongene[b] . W_ng + bias) + (emb[j] . W_e)
with nongene = x[:, G:G+64] and W = [W_ng(64) | w_x(1) | W_e(32)].

Sharding: gene-parallel across 8 cores; each core handles a 2500-gene
column slice for the full 1024-row batch; the tiny fc weights and the
nongene block are replicated.

The kernel is HBM-bandwidth-bound.  Per-core traffic is 8.04 MB
(x gene slice as fp8 e3m4 2.56 MB + y as bf16 5.12 MB + 0.46 MB packed
consts), measured at ~340 GB/s effective against the ~358 GB/s per-NC
HBM limit → ~22.6 us/iteration.  fp8 y output was tried and fails the
2e-2 gate (2.01e-2 in CoreSim: the top-binade half-ulp alone is 1.7e-2
of max|y|).  Compute is far below the DMA floor (ACT 10.3 / Pool 6.4 /
DVE 5.9 us), so everything element-wise stays off the critical path.

DMA plan: 9 DMAs spread over all three DMA queues (qSPDynamicHW /
qActDynamicHW / SWDGE):
  - 1 packed const load on the gpsimd (SWDGE) queue at t=0,
  - 4 x quarter-loads [128, 5000] fp8, alternating sync/act queues,
  - 4 y pair-stores [128, 5000] bf16 split sync/act/gpsimd queues,
    placed at queue positions that never stall a compute engine
    (sync carries stores mid-stream; act/gpsimd only after their
    compute is drained).
x and y use a col-block = row-block packed DRAM layout ([128, 8*2500])
so row blocks are column ranges of one 128-partition tensor (pure host
layout transform, undone in unshard()).

Per-block compute: PE broadcasts w_x ([1,1] matmul vs a ones row),
reduces nongene.W_ng per row block, and builds grow = gene-term + fc
bias on all partitions via the [W_e | b] x [embT ; ones] trick; ACT
does y = Identity(x * w_x + ng[a]) on cols [0:SPLIT), Pool the rest via
tensor_scalar, DVE adds grow (bf16 2x mode).

The repeat/serial knobs exist only for timing: repeat re-issues the
main stream R times inside one NEFF (slope timing); serial adds an
inter-repeat barrier (a strided gate load that RAW-depends on all four
stores, splashed into the next x tiles) so the slope approximates the
full one-shot latency chain (~29 us) instead of the pipelined
steady-state (~23 us).
"""

import numpy as np
import ml_dtypes
from contextlib import ExitStack

import concourse.bass as bass
import concourse.bacc as bacc
import concourse.tile as tile
from concourse import mybir
from concourse.bass_utils import run_bass_kernel_spmd

F32 = mybir.dt.float32
BF16 = mybir.dt.bfloat16
FP8 = mybir.dt.float8e3

NP_BF16 = ml_dtypes.bfloat16
NP_FP8 = ml_dtypes.float8_e3m4

B = 1024
G = 20000
DNG = 64
E = 32
FC_IN = DNG + 1 + E       # 97
NCORES = 8
GC = G // NCORES          # 2500 gene columns per core
PB = 128                  # SBUF partitions
RB = B // PB              # 8 row blocks per core
NQ = 4                    # x load quarters / y store pairs
QW = RB // NQ * GC        # 5000 cols per quarter/pair
SPLIT = 1536              # ACT handles [0:SPLIT), Pool [SPLIT:GC) per block

BANK = 512                # f32 columns per PSUM bank
NBANK = (GC + BANK - 1) // BANK

# const pack layout: [64, CW] bf16
#   rows 0:33, cols 0:GC        = [embT ; ones]
#   rows 0:33, cols GC:GC+PB    = [W_e | b] broadcast
#   rows 0:64, cols CX:CX+B     = xng.T
#   rows 0:64, col  CX+B        = W_ng
#   rows 0:64, col  CX+B+1      = w_x
#   rows 0:64, col  CX+B+2      = 1.0
CX = GC + PB              # 2628
CW = CX + B + 3           # 3655


def build_kernel(nc: bass.Bass, repeat: int = 1, serial: bool = False):
    xgd = nc.dram_tensor("xg", [PB, RB * GC], FP8, kind="ExternalInput").ap()
    cpkd = nc.dram_tensor("cpk", [DNG, CW], BF16, kind="ExternalInput").ap()
    ysd = nc.dram_tensor("ys", [PB, RB * GC], BF16, kind="ExternalOutput").ap()

    with tile.TileContext(nc) as tc, ExitStack() as ctx:
        const = ctx.enter_context(tc.tile_pool(name="const", bufs=1))
        psum = ctx.enter_context(tc.tile_pool(name="psum", bufs=1, space="PSUM"))
        xpool = ctx.enter_context(tc.tile_pool(name="xpool", bufs=NQ))
        ypool = ctx.enter_context(tc.tile_pool(name="ypool", bufs=NQ))

        # ---- dummy activation: hoists LoadActFuncSet to t~0 ----
        zin = const.tile([1, 2], F32)
        nc.gpsimd.memset(zin, 0.0)
        zout = const.tile([1, 2], F32)
        nc.scalar.activation(
            out=zout, in_=zin, func=mybir.ActivationFunctionType.Identity
        )

        # ---- const load on the SWDGE queue; x quarters on the two ----
        # ---- HWDGE queues, two each, so all three queues pull at t=0 ----
        cpk = const.tile([DNG, CW], BF16)
        nc.gpsimd.dma_start(out=cpk, in_=cpkd)
        embT = cpk[0:E + 1, 0:GC]               # [33, 2500] = [embT ; ones]
        web = cpk[0:E + 1, GC:GC + PB]          # [33, 128]  = [W_e | b] bcast
        xngT = cpk[:, CX:CX + B]                # [64, 1024] = xng.T
        wngcol = cpk[:, CX + B:CX + B + 1]      # [64, 1]    = W_ng

        gate = const.tile([1, NQ], BF16)        # serial-mode barrier tile

        def issue_x_loads(first: bool):
            x_ts = []
            for q in range(NQ):
                x_t = xpool.tile([PB, QW], FP8, tag="x")
                x_ts.append(x_t)
            if serial and not first:
                # gate: strided read touching every stored region, then a
                # tiny DVE splash into each x tile corner.  Forces repeat
                # r's loads to wait for r-1's stores (one-shot latency
                # approximation for the repeat-slope measurement).
                nc.sync.dma_start(
                    out=gate, in_=ysd[0:1, QW - 2:RB * GC:QW]
                )
                for q in range(NQ):
                    nc.vector.tensor_copy(x_ts[q][0:1, 0:NQ], gate)
            for q in range(NQ):
                eng = nc.sync if q % 2 == 0 else nc.scalar
                eng.dma_start(
                    out=x_ts[q], in_=xgd[:, q * QW:(q + 1) * QW]
                )
            return x_ts

        x_ts = issue_x_loads(first=True)

        # w_x broadcast across partitions: ones[1,128]^T @ wx[1,1]
        wxp = psum.tile([PB, 1], F32, tag="wx")
        nc.tensor.matmul(
            wxp,
            cpk[0:1, CX + B + 2:CX + B + 3].to_broadcast([1, PB]),
            cpk[0:1, CX + B + 1:CX + B + 2],
            start=True,
            stop=True,
        )
        wxc = const.tile([PB, 1], F32)
        nc.vector.tensor_copy(wxc, wxp)

        # ---- ng term on PE: ngp[p, a] = nongene[a*128+p] . W_ng ----
        ngp = psum.tile([PB, RB], F32, tag="ng")
        for a in range(RB):
            nc.tensor.matmul(
                ngp[:, a:a + 1],
                xngT[:, a * PB:(a + 1) * PB],
                wngcol,
                start=True,
                stop=True,
            )
        ngb = const.tile([PB, RB], F32)
        nc.vector.tensor_copy(ngb, ngp)

        # ---- gene term (+ fc bias): matmul + bf16 copy per bank ----
        grow = const.tile([PB, GC], BF16)
        for q in range(NBANK):
            c0 = q * BANK
            cw = min(BANK, GC - c0)
            gps = psum.tile([PB, BANK], F32, tag=f"g{q}")
            nc.tensor.matmul(
                gps[:, 0:cw],
                web,
                embT[:, c0:c0 + cw],
                start=True,
                stop=True,
            )
            nc.vector.tensor_copy(grow[:, c0:c0 + cw], gps[:, 0:cw])

        # ---- main stream: 8 row blocks in 4 quarter tiles ----
        # Block a lives in x_ts[a//2][:, (a%2)*GC :], y pair tiles hold
        # blocks (2k, 2k+1).  ACT does cols [0:SPLIT), Pool the rest,
        # DVE adds grow.  Stores: pair 0,2 -> sync queue (idle engine),
        # pair 1 -> act queue after its compute, pair 3 -> gpsimd queue
        # after its compute.
        lo = slice(0, SPLIT)
        hi = slice(SPLIT, GC)
        for r in range(repeat):
            if r > 0:
                x_ts = issue_x_loads(first=False)
            y_ts = []
            for k in range(NQ):
                y_t = ypool.tile([PB, QW], BF16, tag="y")
                y_ts.append(y_t)
            # phase 1: per-block scale-add on ACT (lo) and Pool (hi)
            for a in range(RB):
                x_blk = x_ts[a // 2][:, (a % 2) * GC:(a % 2 + 1) * GC]
                y_blk = y_ts[a // 2][:, (a % 2) * GC:(a % 2 + 1) * GC]
                nc.scalar.activation(
                    out=y_blk[:, lo],
                    in_=x_blk[:, lo],
                    func=mybir.ActivationFunctionType.Identity,
                    bias=ngb[:, a:a + 1],
                    scale=wxc,
                )
                nc.gpsimd.tensor_scalar(
                    out=y_blk[:, hi],
                    in0=x_blk[:, hi],
                    scalar1=wxc,
                    scalar2=ngb[:, a:a + 1],
                    op0=mybir.AluOpType.mult,
                    op1=mybir.AluOpType.add,
                )
            # phase 2: grow adds on DVE + stores
            for k in range(NQ):
                y_t = y_ts[k]
                for e in range(2):
                    for sl in (lo, hi):
                        dst = slice(e * GC + sl.start, e * GC + sl.stop)
                        nc.vector.tensor_add(y_t[:, dst], y_t[:, dst], grow[:, sl])
                eng = nc.sync if k % 2 == 0 else (nc.scalar if k == 1 else nc.gpsimd)
                eng.dma_start(out=ysd[:, k * QW:(k + 1) * QW], in_=y_t)


def make_nc(repeat: int = 1, serial: bool = False) -> bacc.Bacc:
    nc = bacc.Bacc("TRN2", debug=False, num_devices=NCORES)
    build_kernel(nc, repeat=repeat, serial=serial)
    nc.compile()
    return nc


def prep_inputs(inputs) -> list:
    """Shard + downcast + repack the full inputs into per-core in_maps."""
    x = np.asarray(inputs["x"], dtype=np.float32)
    emb = np.asarray(inputs["emb"], dtype=np.float32)
    W = np.asarray(inputs["W"], dtype=np.float32).reshape(FC_IN)
    b = float(np.asarray(inputs["b"], dtype=np.float32).reshape(()))

    base = np.zeros((DNG, CW), dtype=np.float32)
    base[:, CX:CX + B] = x[:, G:].T
    base[:, CX + B] = W[0:DNG]
    base[:, CX + B + 1] = W[DNG]
    base[:, CX + B + 2] = 1.0
    base[0:E, GC:GC + PB] = W[DNG + 1:FC_IN, None]
    base[E, GC:GC + PB] = b

    in_maps = []
    for c in range(NCORES):
        sl = slice(c * GC, (c + 1) * GC)
        cpk = base.copy()
        cpk[0:E, 0:GC] = emb[sl].T
        cpk[E, 0:GC] = 1.0
        xg = (
            np.ascontiguousarray(x[:, sl])
            .reshape(RB, PB, GC)
            .transpose(1, 0, 2)
            .reshape(PB, RB * GC)
        )
        in_maps.append({
            "xg": np.ascontiguousarray(xg).astype(NP_FP8),
            "cpk": cpk.astype(NP_BF16),
        })
    return in_maps


def unshard(res_core: np.ndarray) -> np.ndarray:
    """[128, 8*2500] packed -> [1024, 2500] row-major (pure layout)."""
    return (
        np.asarray(res_core)
        .reshape(PB, RB, GC)
        .transpose(1, 0, 2)
        .reshape(B, GC)
        .astype(np.float32)
    )


def kernel(**inputs) -> np.ndarray:
    nc = make_nc()
    in_maps = prep_inputs(inputs)
    res = run_bass_kernel_spmd(nc, in_maps, core_ids=list(range(NCORES)))
    return np.concatenate(
        [unshard(r["ys"]) for r in res.results], axis=1
    )
